# revision 5
# baseline (speedup 1.0000x reference)
"""KNN classification kernel for Trainium2 (8 NeuronCores), v4.

Problem: B=1024 queries x N=200000 gallery, D=256, top-10 neighbors,
softmax-weighted one-hot class scores over 50 classes. The reference's
gallery normalization (per-dim 1/||train[:, d]||) folds into the query
side, so the device only computes q_scaled @ train.T plus a screen.

Device (per core; gallery sharded along N into 8 x 25000, padded to
25088 = 49 "banks" of 512 cols per query-chunk):
  PE:  fp8e4 DoubleRow matmuls (K=256 packed as [128, 2, .]) -> one
       512-col f32 PSUM bank per matmul.
  Screen: PSUM is read exactly once per sim by the only two engines
  with a PSUM port, at 1 elem/lane/cycle each:
   - DVE tensor_reduce(max) over 2-bank (1024-col) spans -> per-256
     maxes (ACT's clock is 1.2 GHz vs DVE 0.96, and ACT's fixed
     per-instruction latency is amortized over 3-bank spans).
   - ACT Relu(x - tau) over 3-bank (1536-col) spans -> fp8 SBUF,
     DMA'd to DRAM; tau is a constant immediate (queries are
     pre-scaled so sigma_dev == 128 for every row; the baked
     threshold uses the min per-query sigma, which only loosens the
     screen).
  PSUM (8 banks) is managed as an explicit ring: each 8-bank cell is
  tiled by a template [A3 A3 D2] / [A3 D3 D2] / [D2 D2 D2 D2] chosen
  greedily to keep both engines' cumulative busy equal; a 1-bank
  startup block gives 8 single-bank warmer spans that bridge the DMA
  pipeline-fill window.
  The loop is gallery-block-major so each gallery DMA block is
  consumed by all 8 query chunks before the next is needed.

Host: flag 256-blocks with DVE max >= tau; take exact candidate
  columns from nonzero relu bytes; rescore all of them exactly in
  f64; exact top-10 -> softmax -> class scores. Certificate: the
  found 10th value must clear tau + 0.33 sigma_dev in device units,
  else that query falls back to a full exact rescore.
Safety: an exact-top-10 item sits at z >= ~3.8 sigma whp while tau is
  ~3.25 sigma and fp8 dot noise is ~0.06 sigma, so a miss needs a
  ~10-sigma-noise deviation; softmax weights are rebuilt from exact
  f64 sims, so screen hits are bit-faithful to the reference.
"""

import os
import numpy as np

NB_KNN = 10
T = 0.07
NUM_CLASSES = 50
EPS = 1e-12

B, N, D = 1024, 200000, 256
NCORES = 8
NPC = N // NCORES           # 25000 real cols per core
BANK = 512
NBK = 49                    # banks per chunk; 49*512 = 25088
NPC_PAD = NBK * BANK
NCH = 8                     # query chunks of 128
SUB = 256                   # DVE max sub-block width
TAU_Z = 3.25                # screen threshold in device-sigma units
CERT_Z = 0.33               # certificate margin in device-sigma units

# gallery DMA blocks in banks: tiny starter block then 6 x 8 so every
# non-starter (block, chunk) cell is exactly one 8-bank PSUM ring line
BLOCKS_NB = [1, 8, 8, 8, 8, 8, 8]

_CACHE = {}


def _acost(nb):
    return nb * BANK / 1.2 + 185.0


def _dcost(nb):
    return nb * BANK / 0.96 + 125.0


# templates for 8-bank cells: list of (nbanks, engine)
_TEMPLATES = [
    [(3, 'A'), (3, 'A'), (2, 'D')],
    [(3, 'A'), (3, 'D'), (2, 'D')],
    [(2, 'D'), (2, 'D'), (2, 'D'), (2, 'D')],
]


def _build_schedule():
    """Greedy global balance of spans onto ACT ('A') and DVE ('D').

    Returns (blocks, cells, spans, rel_total, dve_w):
      blocks: list of (abs_bank0, nbanks)
      cells:  list of dicts: block index, chunk, span index list,
              act width (cols), rel base (cols into rel_d)
      spans:  list of dicts: cell, chunk, abs_bank0, nb, eng,
              psum_bank0, rel_off (cols into the cell's rel tile),
              dve_off (f32 offset into the chunk's dve_o row)
    """
    blocks = []
    b0 = 0
    for nb in BLOCKS_NB:
        blocks.append((b0, nb))
        b0 += nb
    assert b0 == NBK

    tA = tD = 0.0
    cells = []
    spans = []
    rel_total = 0
    dve_off = [0] * NCH
    for bi, (bank0, bnb) in enumerate(blocks):
        for c in range(NCH):
            if bnb == 1:
                lay = [(1, 'A' if tA + _acost(1) <= tD + _dcost(1) else 'D')]
            else:
                best = None
                for cand in _TEMPLATES:
                    a = sum(_acost(n) for n, e in cand if e == 'A')
                    d = sum(_dcost(n) for n, e in cand if e == 'D')
                    key = (max(tA + a, tD + d), tA + a + tD + d)
                    if best is None or key < best[0]:
                        best = (key, cand)
                lay = best[1]
            cell = {"block": bi, "chunk": c, "spans": [],
                    "act_w": 0, "rel_base": rel_total}
            p = 0 if bnb != 1 else c  # singles walk the ring one bank each
            ab = bank0
            for nb, eng in lay:
                sp = {"cell": len(cells), "chunk": c, "bank0": ab,
                      "nb": nb, "eng": eng, "psum0": p,
                      "rel_off": None, "dve_off": None}
                if eng == 'A':
                    tA += _acost(nb)
                    sp["rel_off"] = cell["act_w"]
                    cell["act_w"] += nb * BANK
                else:
                    tD += _dcost(nb)
                    sp["dve_off"] = dve_off[c]
                    dve_off[c] += nb * 2
                cell["spans"].append(len(spans))
                spans.append(sp)
                p += nb
                ab += nb
            rel_total += cell["act_w"]
            cells.append(cell)
    dve_w = max(dve_off)
    return blocks, cells, spans, rel_total, dve_w


BLOCKS, CELLS, SPANS, REL_TOTAL, DVE_W = _build_schedule()


def _build_bass(tau_dev_const):
    import concourse.bacc as bacc
    import concourse.tile as tile
    from concourse import mybir

    nc = bacc.Bacc("TRN2")
    f32 = mybir.dt.float32
    fp8 = mybir.dt.float8e4

    g_d = nc.dram_tensor("g", [128, 2, NPC_PAD], fp8, kind="ExternalInput")
    q_d = nc.dram_tensor("q", [128, 2, B], fp8, kind="ExternalInput")
    dve_d = nc.dram_tensor("dve", [128, NCH * DVE_W], f32,
                           kind="ExternalOutput")
    rel_d = nc.dram_tensor("rel", [128, REL_TOTAL], fp8,
                           kind="ExternalOutput")

    X = mybir.AxisListType.X
    MAX = mybir.AluOpType.max
    RELU = mybir.ActivationFunctionType.Relu
    pm = mybir.MatmulPerfMode.DoubleRow

    max_act_w = max(cell["act_w"] for cell in CELLS)

    with tile.TileContext(nc) as tc:
        with tc.tile_pool(name="qp", bufs=1) as qp, \
             tc.tile_pool(name="gp", bufs=1) as gp, \
             tc.tile_pool(name="op", bufs=1) as op, \
             tc.tile_pool(name="rp", bufs=1) as rp, \
             tc.tile_pool(name="pp", bufs=1, space="PSUM") as pp:
            q8 = qp.tile([128, 2, B], fp8, tag="q8")
            gal = []
            for i, (g0, ng) in enumerate(BLOCKS):
                t = gp.tile([128, 2, ng * BANK], fp8, tag=f"gal{i}",
                            name=f"gal_t{i}")
                gal.append(t)
            dve_o = op.tile([128, NCH * DVE_W], f32, tag="dve_o")
            ps = pp.tile([128, 4096], f32, tag="ps")

            # constant -tau bias, materialized by the otherwise-idle
            # gpsimd engine (no DMA, no HWDGE)
            bias_t = op.tile([128, 1], f32, tag="bias")
            nc.gpsimd.memset(bias_t[:], float(-tau_dev_const))

            # dependency-free warmup activation at t~0 so the implicit
            # act-table load doesn't delay the first real relu
            warm = op.tile([128, 1], f32, tag="warm")
            nc.scalar.activation(out=warm[:], in_=warm[:],
                                 func=RELU, bias=0.0, scale=0.0)

            # startup: q8 chunk 0 first (smallest critical chain), then
            # the 1-bank gallery starter, then the rest of q8 from the
            # DVE queue so the SP/HWDGE issue paths overlap
            nc.sync.dma_start(out=q8[:, :, 0:128], in_=q_d[:, :, 0:128])
            g0b, g0n = BLOCKS[0]
            nc.sync.dma_start(out=gal[0][:],
                              in_=g_d[:, :, g0b * BANK:(g0b + g0n) * BANK])
            nc.gpsimd.dma_start(out=q8[:, :, 128:B], in_=q_d[:, :, 128:B])

            for bi, (bank0, bnb) in enumerate(BLOCKS):
                t = gal[bi]
                if bi + 1 < len(BLOCKS):
                    nb0, nbn = BLOCKS[bi + 1]
                    nc.sync.dma_start(
                        out=gal[bi + 1][:],
                        in_=g_d[:, :, nb0 * BANK:(nb0 + nbn) * BANK])
                for cell in [cl for cl in CELLS if cl["block"] == bi]:
                    c = cell["chunk"]
                    lhs = q8[:, :, c * 128:(c + 1) * 128]
                    rt = None
                    if cell["act_w"]:
                        rt = rp.tile([128, max_act_w], fp8, tag="rel",
                                     bufs=5, name=f"rel_t{cell['block']}_{c}")
                    for si in cell["spans"]:
                        sp = SPANS[si]
                        p0 = sp["psum0"] * BANK
                        w = sp["nb"] * BANK
                        for k in range(sp["nb"]):
                            gcol = (sp["bank0"] + k - bank0) * BANK
                            nc.tensor.matmul(
                                ps[:, p0 + k * BANK:p0 + (k + 1) * BANK],
                                lhs, t[:, :, gcol:gcol + BANK],
                                start=True, stop=True, perf_mode=pm)
                        if sp["eng"] == 'A':
                            ro = sp["rel_off"]
                            nc.scalar.activation(
                                out=rt[:, ro:ro + w], in_=ps[:, p0:p0 + w],
                                func=RELU, bias=bias_t[:, 0:1], scale=1.0)
                        else:
                            do = c * DVE_W + sp["dve_off"]
                            nsub = w // SUB
                            nc.vector.tensor_reduce(
                                dve_o[:, do:do + nsub],
                                ps[:, p0:p0 + w].rearrange(
                                    "p (r w) -> p r w", r=nsub),
                                axis=X, op=MAX)
                    if cell["act_w"]:
                        rb = cell["rel_base"]
                        nc.sync.dma_start(
                            out=rel_d[:, rb:rb + cell["act_w"]],
                            in_=rt[:, :cell["act_w"]])
                    if bi == len(BLOCKS) - 1:
                        h0 = c * DVE_W
                        nc.sync.dma_start(
                            out=dve_d[:, h0:h0 + DVE_W],
                            in_=dve_o[:, h0:h0 + DVE_W])
    if not nc.is_finalized():
        nc.finalize()
    return nc


def _run_device(nc, g_shards, q_packed):
    from concourse.bass_utils import run_bass_kernel_spmd
    in_maps = [{"g": g_shards[c], "q": q_packed} for c in range(NCORES)]
    res = run_bass_kernel_spmd(nc, in_maps, list(range(NCORES)))
    return ([res.results[c]["dve"] for c in range(NCORES)],
            [res.results[c]["rel"] for c in range(NCORES)])


def _run_emulated(g_shards, q_packed, tau_dev_const):
    import ml_dtypes
    FP8 = ml_dtypes.float8_e4m3fn
    qf = q_packed.astype(np.float32)
    dves, rels = [], []
    for core in range(NCORES):
        gf = g_shards[core].astype(np.float32)
        sim = np.einsum("pib,pin->bn", qf, gf)   # [B, NPC_PAD]
        dve = np.zeros((128, NCH * DVE_W), np.float32)
        rel = np.zeros((128, REL_TOTAL), dtype=FP8)
        for cell in CELLS:
            c = cell["chunk"]
            sc = sim[c * 128:(c + 1) * 128]
            for si in cell["spans"]:
                sp = SPANS[si]
                c0 = sp["bank0"] * BANK
                w = sp["nb"] * BANK
                gsl = sc[:, c0:c0 + w]
                if sp["eng"] == 'A':
                    rb = cell["rel_base"] + sp["rel_off"]
                    rel[:, rb:rb + w] = \
                        np.maximum(gsl - tau_dev_const, 0).astype(FP8)
                else:
                    nsub = w // SUB
                    do = c * DVE_W + sp["dve_off"]
                    dve[:, do:do + nsub] = \
                        gsl.reshape(128, nsub, SUB).max(axis=2)
        dves.append(dve)
        rels.append(rel)
    return dves, rels


def kernel(test_features, train_features, train_labels):
    import ml_dtypes
    FP8 = ml_dtypes.float8_e4m3fn

    test_features = np.asarray(test_features, dtype=np.float32)
    train_features = np.asarray(train_features, dtype=np.float32)
    labels = np.asarray(train_labels).astype(np.int64)

    tf64 = train_features.astype(np.float64)
    norm_d = np.maximum(np.sqrt(np.sum(tf64 * tf64, axis=0)), EPS)
    q64 = test_features.astype(np.float64)
    qn = np.sqrt(np.sum(q64 * q64, axis=1, keepdims=True))
    q_scaled = q64 / np.maximum(qn, EPS) / norm_d

    sigma_b = np.sqrt(np.sum(q_scaled * q_scaled, axis=1))
    s_b = 128.0 / sigma_b
    q8 = (q_scaled * s_b[:, None]).astype(FP8)
    g8 = train_features.T.astype(FP8)

    q8f = q8.astype(np.float64)
    g8_sq_mean = float(np.mean(g8.astype(np.float32) ** 2))
    sig_dev = np.sqrt(np.sum(q8f * q8f, axis=1) * g8_sq_mean)
    tau_dev = TAU_Z * sig_dev
    # constant device threshold: the most conservative per-query value
    tau_const = float(tau_dev.min())

    q_packed = np.ascontiguousarray(
        q8.T.reshape(2, 128, B).transpose(1, 0, 2))
    g_shards = []
    for core in range(NCORES):
        sl = np.zeros((2, 128, NPC_PAD), dtype=FP8)
        sl[:, :, :NPC] = g8[:, core * NPC:(core + 1) * NPC].reshape(2, 128, NPC)
        g_shards.append(np.ascontiguousarray(sl.transpose(1, 0, 2)))

    if os.environ.get("KNN_EMULATE"):
        dves, rels = _run_emulated(g_shards, q_packed, tau_const)
    else:
        key = round(tau_const, 6)
        try:
            if key not in _CACHE:
                _CACHE[key] = _build_bass(tau_const)
            nc = _CACHE[key]
            try:
                dves, rels = _run_device(nc, g_shards, q_packed)
            except Exception:
                dves, rels = _run_device(nc, g_shards, q_packed)
        except Exception:
            # last resort: numpy emulation of the device kernel
            dves, rels = _run_emulated(g_shards, q_packed, tau_const)

    # ---- host screen ----
    NBLK = NPC_PAD // SUB
    flags = np.zeros((B, NCORES, NBLK), dtype=bool)
    percol = [[] for _ in range(B)]
    for core in range(NCORES):
        dve = dves[core].astype(np.float64)      # [128, NCH*DVE_W]
        relbytes = np.ascontiguousarray(rels[core]).view(np.uint8)
        for cell in CELLS:
            c = cell["chunk"]
            brow = c * 128
            for si in cell["spans"]:
                sp = SPANS[si]
                if sp["eng"] == 'D':
                    w = sp["nb"] * BANK
                    nsub = w // SUB
                    d0 = c * DVE_W + sp["dve_off"]
                    k0 = (sp["bank0"] * BANK) // SUB
                    m = dve[:, d0:d0 + nsub]
                    flags[brow:brow + 128, core, k0:k0 + nsub] |= \
                        m >= tau_const
            if cell["act_w"]:
                rb = cell["rel_base"]
                seg = relbytes[:, rb:rb + cell["act_w"]]
                nz_p, nz_x = np.nonzero(seg)
                if len(nz_p):
                    # map rel-tile offset back to absolute column: the
                    # cell's A spans are contiguous in rel and ordered
                    # by rel_off
                    loc = np.empty(len(nz_x), np.int64)
                    loc[:] = -1
                    for si in cell["spans"]:
                        sp = SPANS[si]
                        if sp["eng"] != 'A':
                            continue
                        ro = sp["rel_off"]
                        w = sp["nb"] * BANK
                        m = (nz_x >= ro) & (nz_x < ro + w)
                        loc[m] = sp["bank0"] * BANK + (nz_x[m] - ro)
                    valid = (loc >= 0) & (loc < NPC)
                    gcols = core * NPC + loc
                    for p, col in zip(nz_p[valid], gcols[valid]):
                        percol[brow + p].append(col)

    flags = flags.reshape(B, NCORES * NBLK)
    seg_queries = [np.nonzero(flags[:, s])[0] for s in range(NCORES * NBLK)]

    per_q_vals = [[] for _ in range(B)]
    per_q_cols = [[] for _ in range(B)]
    for s, qs in enumerate(seg_queries):
        if len(qs) == 0:
            continue
        core, k = divmod(s, NBLK)
        c0 = core * NPC + SUB * k
        c1 = core * NPC + min(SUB * k + SUB, NPC)
        if c0 >= c1:
            continue
        block = tf64[c0:c1]
        sims = q_scaled[qs] @ block.T
        cols = np.arange(c0, c1)
        for i, b in enumerate(qs):
            per_q_vals[b].append(sims[i])
            per_q_cols[b].append(cols)

    scores = np.zeros((B, NUM_CLASSES), dtype=np.float64)
    fallback = []
    for b in range(B):
        vs = per_q_vals[b]
        cs = per_q_cols[b]
        if percol[b]:
            pc = np.asarray(percol[b], dtype=np.int64)
            vs = vs + [tf64[pc] @ q_scaled[b]]
            cs = cs + [pc]
        if vs:
            v = np.concatenate(vs)
            cidx = np.concatenate(cs)
        else:
            v = np.empty(0)
            cidx = np.empty(0, np.int64)
        if len(v) < NB_KNN:
            fallback.append(b)
            continue
        sel = np.argpartition(-v, NB_KNN - 1)[:NB_KNN]
        v10_dev = s_b[b] * np.sort(v[sel])[0]
        if v10_dev <= tau_const + CERT_Z * sig_dev[b]:
            fallback.append(b)
            continue
        order = np.lexsort((cidx[sel], -v[sel]))
        sel = sel[order]
        topv = v[sel]
        w = np.exp(topv / T - np.max(topv) / T)
        w /= w.sum()
        np.add.at(scores[b], labels[cidx[sel]], w)

    if fallback:
        fb = np.asarray(fallback)
        sims = q_scaled[fb] @ tf64.T
        for i, b in enumerate(fb):
            v = sims[i]
            sel = np.argpartition(-v, NB_KNN - 1)[:NB_KNN]
            order = np.lexsort((sel, -v[sel]))
            sel = sel[order]
            topv = v[sel]
            w = np.exp(topv / T - np.max(topv) / T)
            w /= w.sum()
            np.add.at(scores[b], labels[sel], w)

    return scores.astype(np.float32)


if __name__ == "__main__":
    rng = np.random.default_rng(0)
    tf = rng.standard_normal((B, D), dtype=np.float32)
    trf = rng.standard_normal((N, D), dtype=np.float32)
    trl = rng.integers(0, NUM_CLASSES, N).astype(np.int64)
    os.environ["KNN_EMULATE"] = "1"
    out = kernel(tf, trf, trl)
    print(out.shape, out.dtype, out.sum())


# revision 19
# speedup vs baseline: 1.1507x; 1.1507x over previous
"""KNN classification kernel for Trainium2 (8 NeuronCores), v4.

Problem: B=1024 queries x N=200000 gallery, D=256, top-10 neighbors,
softmax-weighted one-hot class scores over 50 classes. The reference's
gallery normalization (per-dim 1/||train[:, d]||) folds into the query
side, so the device only computes q_scaled @ train.T plus a screen.

Device (per core; gallery sharded along N into 8 x 25000, padded to
25088 = 49 "banks" of 512 cols per query-chunk):
  PE:  fp8e4 DoubleRow matmuls (K=256 packed as [128, 2, .]) -> one
       512-col f32 PSUM bank per matmul.
  Screen: PSUM is read exactly once per sim by the only two engines
  with a PSUM port, at 1 elem/lane/cycle each (DVE 0.96 GHz, ACT
  1.2 GHz):
   - DVE tensor_reduce(max) over 2-bank (1024-col) spans -> per-256
     maxes.
   - ACT Relu(x - tau) over 2-bank spans -> fp8 SBUF, DMA'd to DRAM;
     tau is a constant memset into a bias tile (queries are pre-scaled
     so sigma_dev == 128 for every row; the baked threshold uses the
     min per-query sigma, which only loosens the screen).
  Span size is capped at 2 banks: a span's banks are refilled by
  matmuls only after its read retires, and the refill must fit inside
  (ring lap time - previous drain); 3-bank spans leave a ~680 ns
  window vs the ~690 ns sem+matmul refill chain and stall the ring.
  PSUM (8 banks) is one tile managed as an explicit ring; the ring
  position walks continuously across (gallery block x query chunk)
  cells and a flat span-by-span greedy keeps both engines' cumulative
  busy equal, so the engine interleave stays fine-grained (per-cell
  template quanta would make engine load oscillate beyond what the
  8-bank ring can buffer). A 1-bank startup block gives 8 single-bank
  warmer spans that bridge the DMA pipeline-fill window; later blocks
  are loaded in two half-DMAs so the first half is consumable early
  and gallery transfers never head-of-line-block the rel DMAs.
  The loop is gallery-block-major so each gallery DMA block is
  consumed by all 8 query chunks before the next is needed.

Host: flag 256-blocks with DVE max >= tau; take exact candidate
  columns from nonzero relu bytes; rescore all of them exactly in
  f64; exact top-10 -> softmax -> class scores. Certificate: the
  found 10th value must clear tau + 0.33 sigma_dev in device units,
  else that query falls back to a full exact rescore.
Safety: an exact-top-10 item sits at z >= ~3.8 sigma whp while tau is
  ~3.25 sigma and fp8 dot noise is ~0.06 sigma, so a miss needs a
  ~10-sigma-noise deviation; softmax weights are rebuilt from exact
  f64 sims, so screen hits are bit-faithful to the reference.
"""

import os
import numpy as np

NB_KNN = 10
T = 0.07
NUM_CLASSES = 50
EPS = 1e-12

B, N, D = 1024, 200000, 256
NCORES = 8
NPC = N // NCORES           # 25000 real cols per core
BANK = 512
NBK = 49                    # banks per chunk; 49*512 = 25088
NPC_PAD = NBK * BANK
NCH = 8                     # query chunks of 128
SUB = 256                   # DVE max sub-block width
TAU_Z = 3.25                # screen threshold in device-sigma units
CERT_Z = 0.33               # certificate margin in device-sigma units

# gallery DMA blocks in banks: tiny starter block then 6 x 8 so every
# non-starter (block, chunk) cell is exactly one 8-bank PSUM ring line
BLOCKS_NB = [1, 8, 8, 8, 8, 8, 8]

_CACHE = {}
FIN_BIAS = 0.0              # >0 shifts work toward ACT
A_NB = 2                    # natural ACT span size (banks)
D_NB = 2                    # natural DVE span size (banks)


def _acost(nb):
    return nb * BANK / 1.2 + 185.0


def _dcost(nb):
    return nb * BANK / 0.96 + 125.0


def _build_schedule():
    """Flat span-by-span greedy balance onto ACT ('A') and DVE ('D').

    The PSUM ring position walks continuously across cells so the
    engine interleave stays fine-grained (no per-cell template quanta
    that would make engine load oscillate beyond what the 8-bank ring
    can buffer). Natural span size is 2 banks for both engines; near
    the ring wrap or a cell edge the size is clamped so no span ever
    wraps the 4096-col PSUM boundary.

    Returns (blocks, cells, spans, rel_total, dve_w):
      blocks: list of (abs_bank0, nbanks)
      cells:  list of dicts: block index, chunk, span index list,
              act width (cols), rel base (cols into rel_d)
      spans:  list of dicts: cell, chunk, abs_bank0, nb, eng,
              psum_bank0, rel_off (cols into the cell's rel tile),
              dve_off (f32 offset into the chunk's dve_o row)
    """
    blocks = []
    b0 = 0
    for nb in BLOCKS_NB:
        blocks.append((b0, nb))
        b0 += nb
    assert b0 == NBK

    tA = tD = 0.0
    cells = []
    spans = []
    rel_total = 0
    dve_off = [0] * NCH
    p = 0  # PSUM ring position in banks, walks continuously
    for bi, (bank0, bnb) in enumerate(blocks):
        for c in range(NCH):
            cell = {"block": bi, "chunk": c, "spans": [],
                    "act_w": 0, "rel_base": rel_total}
            left = bnb
            ab = bank0
            last_cell = (bi == len(blocks) - 1 and c == NCH - 1)
            while left:
                r = 8 - p  # banks before the ring wrap
                cap = min(r, left)
                if last_cell and left <= 2:
                    # finish with single-bank spans so both engines can
                    # drain to within one small span of each other
                    cap = 1
                if cap == 1:
                    nb = 1
                    eng = 'A' if tA + _acost(1) + FIN_BIAS <= tD + _dcost(1) else 'D'
                elif cap == 2:
                    nb = 2
                    eng = 'A' if tA + _acost(2) + FIN_BIAS <= tD + _dcost(2) else 'D'
                elif cap == 3 and max(A_NB, D_NB) >= 3:
                    nb = 3
                    eng = 'A' if tA + _acost(3) + FIN_BIAS <= tD + _dcost(3) else 'D'
                else:
                    if tA + _acost(min(A_NB, cap)) + FIN_BIAS <= tD + _dcost(min(D_NB, cap)):
                        eng, nb = 'A', min(A_NB, cap)
                    else:
                        eng, nb = 'D', min(D_NB, cap)
                w = min(nb * BANK, NPC - ab * BANK)  # clip the pad tail
                sp = {"cell": len(cells), "chunk": c, "bank0": ab,
                      "nb": nb, "w": w, "eng": eng, "psum0": p,
                      "nsub": (nb * BANK) // SUB if w == nb * BANK else 1,
                      "rel_off": None, "dve_off": None}
                # nominal (unclipped) costs keep the proven assignment
                # sequence; the clip only shortens the emitted reads
                if eng == 'A':
                    tA += _acost(nb)
                    sp["rel_off"] = cell["act_w"]
                    cell["act_w"] += w
                else:
                    tD += _dcost(nb)
                    sp["dve_off"] = dve_off[c]
                    dve_off[c] += sp["nsub"]
                cell["spans"].append(len(spans))
                spans.append(sp)
                p = (p + nb) % 8
                ab += nb
                left -= nb
            rel_total += cell["act_w"]
            cells.append(cell)
    dve_w = max(dve_off)
    return blocks, cells, spans, rel_total, dve_w


BLOCKS, CELLS, SPANS, REL_TOTAL, DVE_W = _build_schedule()


def _build_bass(tau_dev_const):
    import concourse.bacc as bacc
    import concourse.tile as tile
    from concourse import mybir

    nc = bacc.Bacc("TRN2")
    f32 = mybir.dt.float32
    fp8 = mybir.dt.float8e4

    g_d = nc.dram_tensor("g", [128, 2, NPC_PAD], fp8, kind="ExternalInput")
    q_d = nc.dram_tensor("q", [128, 2, B], fp8, kind="ExternalInput")
    dve_d = nc.dram_tensor("dve", [128, NCH * DVE_W], f32,
                           kind="ExternalOutput")
    rel_d = nc.dram_tensor("rel", [128, REL_TOTAL], fp8,
                           kind="ExternalOutput")

    X = mybir.AxisListType.X
    MAX = mybir.AluOpType.max
    RELU = mybir.ActivationFunctionType.Relu
    pm = mybir.MatmulPerfMode.DoubleRow

    max_act_w = max(cell["act_w"] for cell in CELLS)

    with tile.TileContext(nc) as tc:
        with tc.tile_pool(name="qp", bufs=1) as qp, \
             tc.tile_pool(name="gp", bufs=1) as gp, \
             tc.tile_pool(name="op", bufs=1) as op, \
             tc.tile_pool(name="rp", bufs=1) as rp, \
             tc.tile_pool(name="pp", bufs=1, space="PSUM") as pp:
            q8 = qp.tile([128, 2, B], fp8, tag="q8")
            gal = []
            for i, (g0, ng) in enumerate(BLOCKS):
                t = gp.tile([128, 2, ng * BANK], fp8, tag=f"gal{i}",
                            name=f"gal_t{i}")
                gal.append(t)
            dve_o = op.tile([128, NCH * DVE_W], f32, tag="dve_o")
            ps = pp.tile([128, 4096], f32, tag="ps")

            # constant -tau bias, materialized by the otherwise-idle
            # gpsimd engine (no DMA, no HWDGE)
            bias_t = op.tile([128, 1], f32, tag="bias")
            nc.gpsimd.memset(bias_t[:], float(-tau_dev_const))

            # dependency-free warmup activation at t~0 so the implicit
            # act-table load doesn't delay the first real relu
            warm = op.tile([128, 1], f32, tag="warm")
            nc.scalar.activation(out=warm[:], in_=warm[:],
                                 func=RELU, bias=0.0, scale=0.0)

            # startup: full q8 via SP (one 2KB/partition transfer), the
            # 1-bank gallery starter via the otherwise-idle gpsimd SWDGE
            # path so the two DMA issue pipelines overlap
            nc.sync.dma_start(out=q8[:], in_=q_d[:])
            g0b, g0n = BLOCKS[0]
            nc.gpsimd.dma_start(out=gal[0][:],
                                in_=g_d[:, :, g0b * BANK:(g0b + g0n) * BANK])

            for bi, (bank0, bnb) in enumerate(BLOCKS):
                t = gal[bi]
                if bi + 1 < len(BLOCKS):
                    nb0, nbn = BLOCKS[bi + 1]
                    h = (nbn // 2) * BANK
                    w = nbn * BANK
                    nc.sync.dma_start(
                        out=gal[bi + 1][:, :, 0:h],
                        in_=g_d[:, :, nb0 * BANK:nb0 * BANK + h])
                    nc.sync.dma_start(
                        out=gal[bi + 1][:, :, h:w],
                        in_=g_d[:, :, nb0 * BANK + h:nb0 * BANK + w])
                for cell in [cl for cl in CELLS if cl["block"] == bi]:
                    c = cell["chunk"]
                    lhs = q8[:, :, c * 128:(c + 1) * 128]
                    rt = None
                    if cell["act_w"]:
                        rt = rp.tile([128, max_act_w], fp8, tag="rel",
                                     bufs=5, name=f"rel_t{cell['block']}_{c}")
                    for si in cell["spans"]:
                        sp = SPANS[si]
                        p0 = sp["psum0"] * BANK
                        w = sp["w"]
                        for k in range(sp["nb"]):
                            mw = min(BANK, w - k * BANK)
                            gcol = (sp["bank0"] + k - bank0) * BANK
                            nc.tensor.matmul(
                                ps[:, p0 + k * BANK:p0 + k * BANK + mw],
                                lhs, t[:, :, gcol:gcol + mw],
                                start=True, stop=True, perf_mode=pm)
                        if sp["eng"] == 'A':
                            ro = sp["rel_off"]
                            nc.scalar.activation(
                                out=rt[:, ro:ro + w], in_=ps[:, p0:p0 + w],
                                func=RELU, bias=bias_t[:, 0:1], scale=1.0)
                        else:
                            do = c * DVE_W + sp["dve_off"]
                            nsub = sp["nsub"]
                            nc.vector.tensor_reduce(
                                dve_o[:, do:do + nsub],
                                ps[:, p0:p0 + w].rearrange(
                                    "p (r w) -> p r w", r=nsub),
                                axis=X, op=MAX)
                    if cell["act_w"]:
                        rb = cell["rel_base"]
                        nc.sync.dma_start(
                            out=rel_d[:, rb:rb + cell["act_w"]],
                            in_=rt[:, :cell["act_w"]])
                    if bi == len(BLOCKS) - 1:
                        h0 = c * DVE_W
                        nc.sync.dma_start(
                            out=dve_d[:, h0:h0 + DVE_W],
                            in_=dve_o[:, h0:h0 + DVE_W])
    if not nc.is_finalized():
        nc.finalize()
    return nc


def _run_device(nc, g_shards, q_packed):
    from concourse.bass_utils import run_bass_kernel_spmd
    in_maps = [{"g": g_shards[c], "q": q_packed} for c in range(NCORES)]
    res = run_bass_kernel_spmd(nc, in_maps, list(range(NCORES)))
    return ([res.results[c]["dve"] for c in range(NCORES)],
            [res.results[c]["rel"] for c in range(NCORES)])


def _run_emulated(g_shards, q_packed, tau_dev_const):
    import ml_dtypes
    FP8 = ml_dtypes.float8_e4m3fn
    qf = q_packed.astype(np.float32)
    dves, rels = [], []
    for core in range(NCORES):
        gf = g_shards[core].astype(np.float32)
        sim = np.einsum("pib,pin->bn", qf, gf)   # [B, NPC_PAD]
        dve = np.zeros((128, NCH * DVE_W), np.float32)
        rel = np.zeros((128, REL_TOTAL), dtype=FP8)
        for cell in CELLS:
            c = cell["chunk"]
            sc = sim[c * 128:(c + 1) * 128]
            for si in cell["spans"]:
                sp = SPANS[si]
                c0 = sp["bank0"] * BANK
                w = sp["w"]
                gsl = sc[:, c0:c0 + w]
                if sp["eng"] == 'A':
                    rb = cell["rel_base"] + sp["rel_off"]
                    rel[:, rb:rb + w] = \
                        np.maximum(gsl - tau_dev_const, 0).astype(FP8)
                else:
                    nsub = sp["nsub"]
                    do = c * DVE_W + sp["dve_off"]
                    dve[:, do:do + nsub] = \
                        gsl.reshape(128, nsub, w // nsub).max(axis=2)
        dves.append(dve)
        rels.append(rel)
    return dves, rels


def kernel(test_features, train_features, train_labels):
    import ml_dtypes
    FP8 = ml_dtypes.float8_e4m3fn

    test_features = np.asarray(test_features, dtype=np.float32)
    train_features = np.asarray(train_features, dtype=np.float32)
    labels = np.asarray(train_labels).astype(np.int64)

    tf64 = train_features.astype(np.float64)
    norm_d = np.maximum(np.sqrt(np.sum(tf64 * tf64, axis=0)), EPS)
    q64 = test_features.astype(np.float64)
    qn = np.sqrt(np.sum(q64 * q64, axis=1, keepdims=True))
    q_scaled = q64 / np.maximum(qn, EPS) / norm_d

    sigma_b = np.sqrt(np.sum(q_scaled * q_scaled, axis=1))
    s_b = 128.0 / sigma_b
    q8 = (q_scaled * s_b[:, None]).astype(FP8)
    g8 = train_features.T.astype(FP8)

    q8f = q8.astype(np.float64)
    g8_sq_mean = float(np.mean(g8.astype(np.float32) ** 2))
    sig_dev = np.sqrt(np.sum(q8f * q8f, axis=1) * g8_sq_mean)
    tau_dev = TAU_Z * sig_dev
    # constant device threshold: the most conservative per-query value
    tau_const = float(tau_dev.min())

    q_packed = np.ascontiguousarray(
        q8.T.reshape(2, 128, B).transpose(1, 0, 2))
    g_shards = []
    for core in range(NCORES):
        sl = np.zeros((2, 128, NPC_PAD), dtype=FP8)
        sl[:, :, :NPC] = g8[:, core * NPC:(core + 1) * NPC].reshape(2, 128, NPC)
        g_shards.append(np.ascontiguousarray(sl.transpose(1, 0, 2)))

    if os.environ.get("KNN_EMULATE"):
        dves, rels = _run_emulated(g_shards, q_packed, tau_const)
    else:
        key = round(tau_const, 6)
        try:
            if key not in _CACHE:
                _CACHE[key] = _build_bass(tau_const)
            nc = _CACHE[key]
            try:
                dves, rels = _run_device(nc, g_shards, q_packed)
            except Exception:
                dves, rels = _run_device(nc, g_shards, q_packed)
        except Exception:
            # last resort: numpy emulation of the device kernel
            dves, rels = _run_emulated(g_shards, q_packed, tau_const)

    # ---- host screen ----
    NBLK = NPC_PAD // SUB
    flags = np.zeros((B, NCORES, NBLK), dtype=bool)
    percol = [[] for _ in range(B)]
    for core in range(NCORES):
        dve = dves[core].astype(np.float64)      # [128, NCH*DVE_W]
        relbytes = np.ascontiguousarray(rels[core]).view(np.uint8)
        for cell in CELLS:
            c = cell["chunk"]
            brow = c * 128
            for si in cell["spans"]:
                sp = SPANS[si]
                if sp["eng"] == 'D':
                    w = sp["w"]
                    nsub = sp["nsub"]
                    d0 = c * DVE_W + sp["dve_off"]
                    k0 = (sp["bank0"] * BANK) // SUB
                    m = dve[:, d0:d0 + nsub]
                    if nsub * SUB == w:
                        flags[brow:brow + 128, core, k0:k0 + nsub] |= \
                            m >= tau_const
                    else:
                        # ragged pad-clipped span: one max for the whole
                        # span; conservatively flag every 256-block it
                        # touches
                        nblk = (w + SUB - 1) // SUB
                        flags[brow:brow + 128, core, k0:k0 + nblk] |= \
                            (m >= tau_const).max(axis=1, keepdims=True)
            if cell["act_w"]:
                rb = cell["rel_base"]
                seg = relbytes[:, rb:rb + cell["act_w"]]
                nz_p, nz_x = np.nonzero(seg)
                if len(nz_p):
                    # map rel-tile offset back to absolute column: the
                    # cell's A spans are contiguous in rel and ordered
                    # by rel_off
                    loc = np.empty(len(nz_x), np.int64)
                    loc[:] = -1
                    for si in cell["spans"]:
                        sp = SPANS[si]
                        if sp["eng"] != 'A':
                            continue
                        ro = sp["rel_off"]
                        w = sp["w"]
                        m = (nz_x >= ro) & (nz_x < ro + w)
                        loc[m] = sp["bank0"] * BANK + (nz_x[m] - ro)
                    valid = (loc >= 0) & (loc < NPC)
                    gcols = core * NPC + loc
                    for p, col in zip(nz_p[valid], gcols[valid]):
                        percol[brow + p].append(col)

    flags = flags.reshape(B, NCORES * NBLK)
    seg_queries = [np.nonzero(flags[:, s])[0] for s in range(NCORES * NBLK)]

    per_q_vals = [[] for _ in range(B)]
    per_q_cols = [[] for _ in range(B)]
    for s, qs in enumerate(seg_queries):
        if len(qs) == 0:
            continue
        core, k = divmod(s, NBLK)
        c0 = core * NPC + SUB * k
        c1 = core * NPC + min(SUB * k + SUB, NPC)
        if c0 >= c1:
            continue
        block = tf64[c0:c1]
        sims = q_scaled[qs] @ block.T
        cols = np.arange(c0, c1)
        for i, b in enumerate(qs):
            per_q_vals[b].append(sims[i])
            per_q_cols[b].append(cols)

    scores = np.zeros((B, NUM_CLASSES), dtype=np.float64)
    fallback = []
    for b in range(B):
        vs = per_q_vals[b]
        cs = per_q_cols[b]
        if percol[b]:
            pc = np.asarray(percol[b], dtype=np.int64)
            vs = vs + [tf64[pc] @ q_scaled[b]]
            cs = cs + [pc]
        if vs:
            v = np.concatenate(vs)
            cidx = np.concatenate(cs)
        else:
            v = np.empty(0)
            cidx = np.empty(0, np.int64)
        if len(v) < NB_KNN:
            fallback.append(b)
            continue
        sel = np.argpartition(-v, NB_KNN - 1)[:NB_KNN]
        v10_dev = s_b[b] * np.sort(v[sel])[0]
        if v10_dev <= tau_const + CERT_Z * sig_dev[b]:
            fallback.append(b)
            continue
        order = np.lexsort((cidx[sel], -v[sel]))
        sel = sel[order]
        topv = v[sel]
        w = np.exp(topv / T - np.max(topv) / T)
        w /= w.sum()
        np.add.at(scores[b], labels[cidx[sel]], w)

    if fallback:
        fb = np.asarray(fallback)
        sims = q_scaled[fb] @ tf64.T
        for i, b in enumerate(fb):
            v = sims[i]
            sel = np.argpartition(-v, NB_KNN - 1)[:NB_KNN]
            order = np.lexsort((sel, -v[sel]))
            sel = sel[order]
            topv = v[sel]
            w = np.exp(topv / T - np.max(topv) / T)
            w /= w.sum()
            np.add.at(scores[b], labels[sel], w)

    return scores.astype(np.float32)


if __name__ == "__main__":
    rng = np.random.default_rng(0)
    tf = rng.standard_normal((B, D), dtype=np.float32)
    trf = rng.standard_normal((N, D), dtype=np.float32)
    trl = rng.integers(0, NUM_CLASSES, N).astype(np.int64)
    os.environ["KNN_EMULATE"] = "1"
    out = kernel(tf, trf, trl)
    print(out.shape, out.dtype, out.sum())
